# revision 14
# baseline (speedup 1.0000x reference)
"""Multi-head self-attention (B=2, S=2048, D=1024, H=16, causal) on 8 NeuronCores.

Sharding: core c = 4*b + g handles batch b and heads 4g..4g+3 (batch x
head-group parallel). Per core:
  - q/k projections in transposed layout  qT/kT [dh, s]  (dh on partitions)
  - v projection in natural layout [s, dh] with a fused ones-column per head
    (gives the softmax denominator for free during the AV matmul)
  - causal attention in scoresT [j, i] orientation: PE scores -> ACT exp
    (scale=1/8, no max subtraction; scores ~ N(0,1) so exp never overflows)
    -> DVE causal mask multiply on diagonal blocks -> PE AV accumulate.
    Diagonal j-chunks stream only the i-suffix they can influence.
  - normalization of attnT by the per-query denominator: DVE copy+reciprocal,
    GPSIMD partition-broadcast, DVE multiply during PSUM eviction
  - partial o-projection out_c = merged_c @ Wo[:, cols_c].T
Host sums the 4 partial outputs per batch (the only cross-core reduction).

The emission schedule interleaves projection / o-projection / v work into
the attention stream at sub-group granularity so the in-order PE queue
always has independent work while ACT drains the exp backlog. All PSUM
evictions in the attention phase run on DVE (+GPSIMD for the denominator
broadcast); ACT does only exps and projection-phase copies.

Data path is bf16 (inputs cast on host); PSUM accumulation is fp32.
"""

import numpy as np
import ml_dtypes

import concourse.bass as bass
from concourse import bacc
import concourse.mybir as mybir
import concourse.tile as tile
from concourse import bass_utils

F32 = mybir.dt.float32
F16 = mybir.dt.float16
BF16 = mybir.dt.bfloat16
EXP = mybir.ActivationFunctionType.Exp

B, S, D = 2, 2048, 1024
H, DH = 16, 64
NCORES = 8
HPG = 4                  # heads per group (per core)
M = HPG * DH             # 256 per-core head dims
DC = D // 128            # 8 contraction chunks for projections
IC = 512                 # i (query) chunk for attention
JC = 128                 # j (key) chunk for attention
SCALE = 1.0 / np.sqrt(DH)


def _build_nc():
    nc = bacc.Bacc("TRN2", target_bir_lowering=False, debug=False)

    xT_d = nc.dram_tensor("xT", [D, S], BF16, kind="ExternalInput").ap()
    wqkv_d = nc.dram_tensor("wqkvT", [D, 3 * M], BF16, kind="ExternalInput").ap()
    woT_d = nc.dram_tensor("woT", [M, D], BF16, kind="ExternalInput").ap()
    mask_d = nc.dram_tensor("mask", [JC, 768], BF16, kind="ExternalInput").ap()
    onesb_d = nc.dram_tensor("ones_b", [JC, HPG], BF16, kind="ExternalInput").ap()
    out_d = nc.dram_tensor("out", [S, D], F16, kind="ExternalOutput").ap()

    with tile.TileContext(nc) as tc:
        _body(tc, xT_d, wqkv_d, woT_d, mask_d, onesb_d, out_d)
    nc.compile()
    return nc


def _body(tc, xT_d, wqkv_d, woT_d, mask_d, onesb_d, out_d):
    nc = tc.nc
    from contextlib import ExitStack
    ctx = ExitStack()
    with ctx:
        p_x = ctx.enter_context(tc.tile_pool(name="x", bufs=DC))
        p_w = ctx.enter_context(tc.tile_pool(name="w", bufs=DC))
        p_wo = ctx.enter_context(tc.tile_pool(name="wo", bufs=2))
        p_qk = ctx.enter_context(tc.tile_pool(name="qk", bufs=2))
        p_v = ctx.enter_context(tc.tile_pool(name="v", bufs=S // JC))
        p_mg = ctx.enter_context(tc.tile_pool(name="mg", bufs=2))
        p_probs = ctx.enter_context(tc.tile_pool(name="probs", bufs=8))
        p_small = ctx.enter_context(tc.tile_pool(name="small", bufs=2))
        p_mask = ctx.enter_context(tc.tile_pool(name="mask", bufs=1))
        p_ostg = ctx.enter_context(tc.tile_pool(name="ostg", bufs=2))
        p_ones = ctx.enter_context(tc.tile_pool(name="ones", bufs=1))

        ps_big = ctx.enter_context(tc.tile_pool(name="psb", bufs=2, space="PSUM"))
        ps_sc = ctx.enter_context(tc.tile_pool(name="pss", bufs=2, space="PSUM"))
        ps_at = ctx.enter_context(tc.tile_pool(name="psa", bufs=2, space="PSUM"))

        # ---- HAM pre-warm: the PE idles waiting for the first x/w tiles
        # anyway; a burst of discarded fp32 matmuls keeps the activity
        # monitor busy so the clock gate is at full rate when the real
        # projections start.
        wrm = p_ones.tile([128, 512], F32, tag="warm")
        nc.vector.memset(wrm[:], 1.0)
        wrmb = p_ones.tile([128, 512], BF16, tag="warmb")
        nc.vector.memset(wrmb[:], 1.0)
        wrm_ps = ps_at.tile([128, 512], F32, tag="attn", name="warmps")
        for r in range(9):
            nc.tensor.matmul(wrm_ps[:], wrm[:, 0:128], wrm[:],
                             start=(r == 0), stop=(r == 8))
        nc.scalar.copy(wrm[:, 0:1], wrm_ps[:, 0:1])  # keep alive vs DCE

        # ---- input loads, in consumption order
        w_t, x_t = [], []
        for dc in range(DC):
            eng = nc.gpsimd if dc < 2 else nc.sync
            wt = p_w.tile([128, 3 * M], BF16, tag="w")
            eng.dma_start(wt[:], wqkv_d[dc * 128:(dc + 1) * 128, :])
            w_t.append(wt)
            xt = p_x.tile([128, S], BF16, tag="x")
            eng.dma_start(xt[:], xT_d[dc * 128:(dc + 1) * 128, :])
            x_t.append(xt)
        wo_t = []
        for kc in range(2):
            t = p_wo.tile([128, D], BF16, tag="wo")
            nc.sync.dma_start(t[:], woT_d[kc * 128:(kc + 1) * 128, :])
            wo_t.append(t)
        mask_t = p_mask.tile([JC, 768], BF16, tag="mask")
        nc.sync.dma_start(mask_t[:], mask_d[:])
        onesb_t = p_ones.tile([JC, HPG], BF16, tag="onesb")
        nc.sync.dma_start(onesb_t[:], onesb_d[:])

        # ---- projections ----
        q_t, k_t = {}, {}

        def qk_proj0():
            # m-chunk 0 runs while the input DMA is still streaming: the
            # d-contraction is split in two half-groups (dc 0-3, dc 4-7)
            # merged at eviction, so the in-order PE never stalls waiting
            # for the last x tiles.
            for woff, store, tg in ((0, q_t, "qT"), (M, k_t, "kT")):
                dst = p_qk.tile([128, S], BF16, tag=tg, name=f"{tg}0")
                for s4 in range(S // 512):
                    sl = slice(s4 * 512, (s4 + 1) * 512)
                    psa = ps_big.tile([128, 512], F32, tag="proj")
                    for dc in range(DC // 2):
                        nc.tensor.matmul(
                            psa[:], w_t[dc][:, woff:woff + 128], x_t[dc][:, sl],
                            start=(dc == 0), stop=(dc == DC // 2 - 1))
                    psb = ps_sc.tile([128, 2 * IC], F32, tag="scores")
                    for dc in range(DC // 2, DC):
                        nc.tensor.matmul(
                            psb[:, 0:512], w_t[dc][:, woff:woff + 128],
                            x_t[dc][:, sl],
                            start=(dc == DC // 2), stop=(dc == DC - 1))
                    nc.scalar.copy(dst[:, sl], psa[:])
                    nc.vector.tensor_add(dst[:, sl], dst[:, sl], psb[:, 0:512])
                store[0] = dst

        def qk1_unit(which, s4):
            # m-chunk 1, one (q|k, s4) block: inputs have long arrived, so a
            # single 8-deep accumulation chain + one ACT eviction suffices.
            woff, store, tg = ((0, q_t, "qT") if which == "q" else (M, k_t, "kT"))
            if 1 not in store:
                store[1] = p_qk.tile([128, S], BF16, tag=tg, name=f"{tg}1")
            dst = store[1]
            sl = slice(s4 * 512, (s4 + 1) * 512)
            psa = ps_big.tile([128, 512], F32, tag="proj")
            for dc in range(DC):
                nc.tensor.matmul(
                    psa[:], w_t[dc][:, woff + 128:woff + 256], x_t[dc][:, sl],
                    start=(dc == 0), stop=(dc == DC - 1))
            nc.scalar.copy(dst[:, sl], psa[:])

        v_t = {}

        def v_proj(sc, split):
            # v[s, m] tile for j-chunk sc: per head h cols h*65..h*65+63 = v,
            # col h*65+64 = 1.0 (softmax denominator column).
            vt = p_v.tile([JC, HPG * (DH + 1)], BF16, tag="v", name=f"v{sc}")
            nc.vector.tensor_copy(
                vt[:].rearrange("p (h e) -> p h e", h=HPG)[:, :, DH:DH + 1].squeeze(2),
                onesb_t[:])
            dstv = vt[:].rearrange("p (h e) -> p h e", h=HPG)[:, :, 0:DH]
            if split:
                psa = ps_big.tile([128, 512], F32, tag="proj")
                for dc in range(DC // 2):
                    nc.tensor.matmul(
                        psa[:, 0:M], x_t[dc][:, sc * 128:(sc + 1) * 128],
                        w_t[dc][:, 2 * M:3 * M],
                        start=(dc == 0), stop=(dc == DC // 2 - 1))
                psb = ps_sc.tile([128, 2 * IC], F32, tag="scores")
                for dc in range(DC // 2, DC):
                    nc.tensor.matmul(
                        psb[:, 0:M], x_t[dc][:, sc * 128:(sc + 1) * 128],
                        w_t[dc][:, 2 * M:3 * M],
                        start=(dc == DC // 2), stop=(dc == DC - 1))
                nc.scalar.activation(
                    dstv, psa[:, 0:M].rearrange("p (h d) -> p h d", h=HPG),
                    mybir.ActivationFunctionType.Copy)
                nc.vector.tensor_add(
                    dstv, dstv, psb[:, 0:M].rearrange("p (h d) -> p h d", h=HPG))
            else:
                psa = ps_big.tile([128, 512], F32, tag="proj")
                for dc in range(DC):
                    nc.tensor.matmul(
                        psa[:, 0:M], x_t[dc][:, sc * 128:(sc + 1) * 128],
                        w_t[dc][:, 2 * M:3 * M],
                        start=(dc == 0), stop=(dc == DC - 1))
                nc.vector.tensor_copy(
                    dstv, psa[:, 0:M].rearrange("p (h d) -> p h d", h=HPG))
            v_t[sc] = vt

        # ---- attention ----
        mg_t = [p_mg.tile([128, S], BF16, tag="mgT", name=f"mg{i}")
                for i in range(M // 128)]
        gstate = {}  # (h, ic) -> {"at": psum tile, "prs": [(pr, ja, jb)]}

        def gs(h, ic):
            return gstate.setdefault((h, ic), {"at": None, "prs": []})

        def at_tile(g):
            if g["at"] is None:
                g["at"] = ps_at.tile([DH + 1, IC], F32, tag="attn", name="atps")
            return g["at"]

        def sc_block(h, ic, pa, pb):
            # scores+exp for non-diagonal pairs pa..pb-1 of group (h, ic)
            g = gs(h, ic)
            qk_tile, prow = h // 2, 64 * (h % 2)
            kk, qq = k_t[qk_tile], q_t[qk_tile]
            i0 = ic * IC
            for p in range(pa, pb):
                ja, jb = 2 * p, 2 * p + 2
                sc_ps = ps_sc.tile([128, 2 * IC], F32, tag="scores")
                pr = p_probs.tile([JC, 2 * IC], BF16, tag="probs")
                for u, jc in enumerate(range(ja, jb)):
                    nc.tensor.matmul(
                        sc_ps[:, u * IC:(u + 1) * IC],
                        kk[prow:prow + DH, jc * JC:(jc + 1) * JC],
                        qq[prow:prow + DH, i0:i0 + IC],
                        start=True, stop=True)
                nc.scalar.activation(pr[:], sc_ps[:], EXP, scale=SCALE)
                g["prs"].append((pr, ja, jb))

        def av_block(h, ic):
            # consume all pending probs of group (h, ic)
            g = gs(h, ic)
            at_ps = at_tile(g)
            vsl = slice(h * (DH + 1), (h + 1) * (DH + 1))
            for pr, ja, jb in g["prs"]:
                for u, jc in enumerate(range(ja, jb)):
                    nc.tensor.matmul(
                        at_ps[:], v_t[jc][:, vsl], pr[:, u * IC:(u + 1) * IC],
                        start=(jc == 0), stop=False)
            g["prs"] = []

        def diag_scores(h, ic):
            # diagonal chunks nd..nd+3: d0 full 512 stream, d128 384 (cols
            # 512:896 of pair A), d256/d384 256 each (pair B cols 0:512).
            g = gs(h, ic)
            qk_tile, prow = h // 2, 64 * (h % 2)
            kk, qq = k_t[qk_tile], q_t[qk_tile]
            i0 = ic * IC
            nd = 4 * ic
            psA = ps_sc.tile([128, 2 * IC], F32, tag="scores")
            prA = p_probs.tile([JC, 2 * IC], BF16, tag="probs")
            nc.tensor.matmul(
                psA[:, 0:512], kk[prow:prow + DH, nd * JC:(nd + 1) * JC],
                qq[prow:prow + DH, i0:i0 + IC], start=True, stop=True)
            nc.tensor.matmul(
                psA[:, 512:896], kk[prow:prow + DH, (nd + 1) * JC:(nd + 2) * JC],
                qq[prow:prow + DH, i0 + 128:i0 + IC], start=True, stop=True)
            psB = ps_sc.tile([128, 2 * IC], F32, tag="scores")
            prB = p_probs.tile([JC, 2 * IC], BF16, tag="probs")
            nc.tensor.matmul(
                psB[:, 0:256], kk[prow:prow + DH, (nd + 2) * JC:(nd + 3) * JC],
                qq[prow:prow + DH, i0 + 256:i0 + IC], start=True, stop=True)
            nc.tensor.matmul(
                psB[:, 256:512], kk[prow:prow + DH, (nd + 3) * JC:(nd + 4) * JC],
                qq[prow:prow + DH, i0 + 256:i0 + IC], start=True, stop=True)
            nc.scalar.activation(prA[:, 0:896], psA[:, 0:896], EXP, scale=SCALE)
            nc.scalar.activation(prB[:, 0:512], psB[:, 0:512], EXP, scale=SCALE)
            pvA = prA[:].rearrange("p (a b) -> p a b", a=2)[:, :, 0:128]
            nc.vector.tensor_mul(
                pvA, pvA, mask_t[:, 0:256].rearrange("p (a b) -> p a b", a=2))
            nc.vector.tensor_mul(prB[:, 0:512], prB[:, 0:512], mask_t[:, 256:768])
            g["diag"] = (prA, prB)

        def diag_avs(h, ic):
            g = gs(h, ic)
            at_ps = at_tile(g)
            prA, prB = g.pop("diag")
            vsl = slice(h * (DH + 1), (h + 1) * (DH + 1))
            nd = 4 * ic
            nc.tensor.matmul(at_ps[:], v_t[nd][:, vsl], prA[:, 0:512],
                             start=(nd == 0), stop=False)
            nc.tensor.matmul(at_ps[:, 128:512], v_t[nd + 1][:, vsl],
                             prA[:, 512:896], start=False, stop=False)
            nc.tensor.matmul(at_ps[:, 256:512], v_t[nd + 2][:, vsl],
                             prB[:, 0:256], start=False, stop=False)
            nc.tensor.matmul(at_ps[:, 256:512], v_t[nd + 3][:, vsl],
                             prB[:, 256:512], start=False, stop=True)

        def normalize(h, ic):
            # DVE copies the denominator row to SBUF, reciprocates it,
            # GPSIMD broadcasts the reciprocal across 64 partitions, DVE
            # multiplies during the attnT eviction into mergedT.
            g = gstate.pop((h, ic))
            at_ps = g["at"]
            qk_tile, prow = h // 2, 64 * (h % 2)
            den = p_small.tile([1, IC], F32, tag="den")
            nc.vector.tensor_copy(den[:], at_ps[DH:DH + 1, :])
            rc32 = p_small.tile([1, IC], F32, tag="recip32")
            nc.vector.reciprocal_approx_fast(rc32[:], den[:])
            bc_sb = p_small.tile([DH, IC], F32, tag="bcast")
            nc.gpsimd.partition_broadcast(bc_sb[:], rc32[:])
            nc.vector.tensor_mul(
                mg_t[qk_tile][prow:prow + DH, ic * IC:(ic + 1) * IC],
                at_ps[0:DH, :], bc_sb[:])

        def oproj(sc):
            # out[s, o] = sum_k mergedT[k, s] woT[k, o]; both half-evictions
            # on DVE (ACT is saturated with exps in the attention phase).
            stg = p_ostg.tile([128, D], F16, tag="ostg")
            for nn in range(2):
                ps = ps_big.tile([128, 512], F32, tag="proj")
                for kc in range(2):
                    nc.tensor.matmul(
                        ps[:], mg_t[kc][:, sc * 128:(sc + 1) * 128],
                        wo_t[kc][:, nn * 512:(nn + 1) * 512],
                        start=(kc == 0), stop=(kc == 1))
                nc.vector.tensor_copy(stg[:, nn * 512:(nn + 1) * 512], ps[:])
            nc.sync.dma_start(out_d[sc * 128:(sc + 1) * 128, :], stg[:])

        # ---- emission schedule ----
        # ic=0 groups are diagonal-only; qk m-chunk 1 units fill the PE
        # stream between them. v chunks and o-projection blocks are woven
        # into later attention groups so the PE always has independent work
        # queued while ACT drains the exp backlog of the previous block.
        qk_proj0()
        for sc in range(4):
            v_proj(sc, split=True)

        OPS = [
            ("dgs", 0, 0), ("dga", 0, 0),
            ("q1", "q", 0),
            ("dgs", 1, 0), ("dga", 1, 0),
            ("nrm", 0, 0),
            ("q1", "q", 1), ("q1", "q", 2), ("q1", "q", 3),
            ("nrm", 1, 0),
            ("q1", "k", 0), ("q1", "k", 1), ("q1", "k", 2), ("q1", "k", 3),
            ("dgs", 2, 0), ("dga", 2, 0),
            ("v", 4),
            ("dgs", 3, 0), ("dga", 3, 0),
            ("nrm", 2, 0),
            ("v", 5),
            ("nrm", 3, 0),
            ("op", 0), ("v", 6), ("op", 1), ("v", 7), ("op", 2), ("op", 3),
            # ---- ic = 1: one 2-pair block + diagonal per group ----
            ("scb", 0, 1, 0, 2), ("v", 8), ("avb", 0, 1), ("dgs", 0, 1), ("dga", 0, 1),
            ("scb", 1, 1, 0, 2), ("v", 9), ("avb", 1, 1), ("dgs", 1, 1), ("dga", 1, 1),
            ("nrm", 0, 1),
            ("scb", 2, 1, 0, 2), ("v", 10), ("avb", 2, 1), ("dgs", 2, 1), ("dga", 2, 1),
            ("nrm", 1, 1),
            ("scb", 3, 1, 0, 2), ("v", 11), ("avb", 3, 1), ("dgs", 3, 1), ("dga", 3, 1),
            ("nrm", 2, 1),
            # ---- ic = 2: two 2-pair blocks + diagonal per group ----
            ("scb", 0, 2, 0, 2), ("nrm", 3, 1), ("avb", 0, 2),
            ("scb", 0, 2, 2, 4), ("v", 12), ("avb", 0, 2), ("dgs", 0, 2), ("dga", 0, 2),
            ("scb", 1, 2, 0, 2), ("v", 13), ("avb", 1, 2),
            ("scb", 1, 2, 2, 4), ("v", 14), ("avb", 1, 2), ("dgs", 1, 2), ("dga", 1, 2),
            ("nrm", 0, 2),
            ("scb", 2, 2, 0, 2), ("v", 15), ("avb", 2, 2),
            ("scb", 2, 2, 2, 4), ("op", 4), ("avb", 2, 2), ("dgs", 2, 2), ("dga", 2, 2),
            ("nrm", 1, 2),
            ("scb", 3, 2, 0, 2), ("op", 5), ("avb", 3, 2),
            ("scb", 3, 2, 2, 4), ("op", 6), ("avb", 3, 2), ("dgs", 3, 2), ("dga", 3, 2),
            ("nrm", 2, 2),
            # ---- ic = 3: three 2-pair blocks + diagonal per group ----
            ("scb", 0, 3, 0, 2), ("op", 7), ("avb", 0, 3),
            ("scb", 0, 3, 2, 4), ("nrm", 3, 2), ("avb", 0, 3),
            ("scb", 0, 3, 4, 6), ("avb", 0, 3), ("dgs", 0, 3), ("dga", 0, 3),
            ("scb", 1, 3, 0, 2), ("avb", 1, 3),
            ("scb", 1, 3, 2, 4), ("avb", 1, 3),
            ("scb", 1, 3, 4, 6), ("avb", 1, 3), ("dgs", 1, 3), ("dga", 1, 3),
            ("nrm", 0, 3),
            ("scb", 2, 3, 0, 2), ("op", 8), ("avb", 2, 3),
            ("scb", 2, 3, 2, 4), ("op", 9), ("avb", 2, 3),
            ("scb", 2, 3, 4, 6), ("op", 10), ("avb", 2, 3), ("dgs", 2, 3), ("dga", 2, 3),
            ("nrm", 1, 3),
            ("scb", 3, 3, 0, 2), ("op", 11), ("avb", 3, 3),
            ("scb", 3, 3, 2, 4), ("nrm", 2, 3), ("avb", 3, 3),
            ("scb", 3, 3, 4, 6), ("avb", 3, 3), ("dgs", 3, 3), ("dga", 3, 3),
            ("twarm",), ("nrm", 3, 3),
            ("op", 12), ("op", 13), ("op", 14), ("op", 15),
        ]
        for op in OPS:
            kind = op[0]
            if kind == "q1":
                qk1_unit(op[1], op[2])
            elif kind == "v":
                v_proj(op[1], split=False)
            elif kind == "scb":
                sc_block(op[1], op[2], op[3], op[4])
            elif kind == "avb":
                av_block(op[1], op[2])
            elif kind == "dgs":
                diag_scores(op[1], op[2])
            elif kind == "dga":
                diag_avs(op[1], op[2])
            elif kind == "nrm":
                normalize(op[1], op[2])
            elif kind == "op":
                oproj(op[1])
            elif kind == "twarm":
                # dummy burst bridging the final normalize's latency window:
                # keeps the HAM activity monitor fed so the closing
                # o-projections run at full clock instead of k=4/8.
                twps = ps_at.tile([128, 512], F32, tag="attn", name="twps")
                for r in range(10):
                    nc.tensor.matmul(twps[:], wrmb[:, 0:128], wrmb[:],
                                     start=(r == 0), stop=(r == 9))
                nc.scalar.copy(wrm[:, 1:2], twps[:, 0:1])


_NC_CACHE = None


def _get_nc():
    global _NC_CACHE
    if _NC_CACHE is None:
        _NC_CACHE = _build_nc()
    return _NC_CACHE


def _causal_mask_tile():
    # [128, 768]: cols 0:128 and 128:256 = tri(j <= u) (masks the first 128
    # trimmed columns of the d0 and d128 diagonal chunks via one strided
    # multiply); cols 256:512 = tri(j <= u) over u in [0,256) (d256); cols
    # 512:768 = tri(j <= u - 128) (d384).
    j = np.arange(JC)[:, None]
    u128 = np.arange(128)[None, :]
    u256 = np.arange(256)[None, :]
    tri128 = (j <= u128).astype(np.float32)
    triB1 = (j <= u256).astype(np.float32)
    triB2 = (j <= u256 - 128).astype(np.float32)
    return np.concatenate([tri128, tri128, triB1, triB2], axis=1)


def _prepare_in_maps(inputs):
    bf = ml_dtypes.bfloat16
    x = np.asarray(inputs["in_features"], dtype=np.float32)
    wqT = np.ascontiguousarray(np.asarray(inputs["q_proj_weight"], np.float32).T)
    wkT = np.ascontiguousarray(np.asarray(inputs["k_proj_weight"], np.float32).T)
    wvT = np.ascontiguousarray(np.asarray(inputs["v_proj_weight"], np.float32).T)
    woT = np.ascontiguousarray(np.asarray(inputs["o_proj_weight"], np.float32).T)
    xT = [np.ascontiguousarray(x[b].T) for b in range(B)]
    mask = _causal_mask_tile()

    in_maps = []
    for c in range(NCORES):
        b, g = divmod(c, HPG)
        ms = slice(g * M, (g + 1) * M)
        in_maps.append({
            "xT": xT[b].astype(bf),
            "wqkvT": np.ascontiguousarray(
                np.concatenate([wqT[:, ms], wkT[:, ms], wvT[:, ms]], axis=1)).astype(bf),
            "woT": np.ascontiguousarray(woT[ms, :]).astype(bf),
            "mask": mask.astype(bf),
            "ones_b": np.ones((JC, HPG), bf),
        })
    return in_maps


def kernel(q_proj_weight, k_proj_weight, v_proj_weight, o_proj_weight, in_features):
    in_dtype = np.asarray(in_features).dtype
    in_maps = _prepare_in_maps({
        "q_proj_weight": q_proj_weight,
        "k_proj_weight": k_proj_weight,
        "v_proj_weight": v_proj_weight,
        "o_proj_weight": o_proj_weight,
        "in_features": in_features,
    })
    nc = _get_nc()
    res = bass_utils.run_bass_kernel_spmd(nc, in_maps, core_ids=list(range(NCORES)))
    out = np.zeros((B, S, D), dtype=np.float32)
    for c in range(NCORES):
        out[c // HPG] += res.results[c]["out"]
    return out.astype(in_dtype)


# revision 15
# speedup vs baseline: 1.0037x; 1.0037x over previous
"""Multi-head self-attention (B=2, S=2048, D=1024, H=16, causal) on 8 NeuronCores.

Sharding: core c = 4*b + g handles batch b and heads 4g..4g+3 (batch x
head-group parallel). Per core:
  - q/k projections in transposed layout  qT/kT [dh, s]  (dh on partitions)
  - v projection in natural layout [s, dh] with a fused ones-column per head
    (gives the softmax denominator for free during the AV matmul)
  - causal attention in scoresT [j, i] orientation: PE scores -> ACT exp
    (scale=1/8, no max subtraction; scores ~ N(0,1) so exp never overflows)
    -> DVE causal mask multiply on diagonal blocks -> PE AV accumulate.
    Diagonal j-chunks stream only the i-suffix they can influence.
  - normalization of attnT by the per-query denominator: DVE copy+reciprocal,
    GPSIMD partition-broadcast, DVE multiply during PSUM eviction
  - partial o-projection out_c = merged_c @ Wo[:, cols_c].T
Host sums the 4 partial outputs per batch (the only cross-core reduction).

The emission schedule interleaves projection / o-projection / v work into
the attention stream at sub-group granularity so the in-order PE queue
always has independent work while ACT drains the exp backlog. All PSUM
evictions in the attention phase run on DVE (+GPSIMD for the denominator
broadcast); ACT does only exps and projection-phase copies.

Data path is bf16 (inputs cast on host); PSUM accumulation is fp32.
"""

import numpy as np
import ml_dtypes

import concourse.bass as bass
from concourse import bacc
import concourse.mybir as mybir
import concourse.tile as tile
from concourse import bass_utils

F32 = mybir.dt.float32
F16 = mybir.dt.float16
BF16 = mybir.dt.bfloat16
EXP = mybir.ActivationFunctionType.Exp

B, S, D = 2, 2048, 1024
H, DH = 16, 64
NCORES = 8
HPG = 4                  # heads per group (per core)
M = HPG * DH             # 256 per-core head dims
DC = D // 128            # 8 contraction chunks for projections
IC = 512                 # i (query) chunk for attention
JC = 128                 # j (key) chunk for attention
SCALE = 1.0 / np.sqrt(DH)


def _build_nc():
    nc = bacc.Bacc("TRN2", target_bir_lowering=False, debug=False)

    xT_d = nc.dram_tensor("xT", [D, S], BF16, kind="ExternalInput").ap()
    wqkv_d = nc.dram_tensor("wqkvT", [D, 3 * M], BF16, kind="ExternalInput").ap()
    woT_d = nc.dram_tensor("woT", [M, D], BF16, kind="ExternalInput").ap()
    mask_d = nc.dram_tensor("mask", [JC, 768], BF16, kind="ExternalInput").ap()
    onesb_d = nc.dram_tensor("ones_b", [JC, HPG], BF16, kind="ExternalInput").ap()
    out_d = nc.dram_tensor("out", [S, D], F16, kind="ExternalOutput").ap()

    with tile.TileContext(nc) as tc:
        _body(tc, xT_d, wqkv_d, woT_d, mask_d, onesb_d, out_d)
    nc.compile()
    return nc


def _body(tc, xT_d, wqkv_d, woT_d, mask_d, onesb_d, out_d):
    nc = tc.nc
    from contextlib import ExitStack
    ctx = ExitStack()
    with ctx:
        p_x = ctx.enter_context(tc.tile_pool(name="x", bufs=DC))
        p_w = ctx.enter_context(tc.tile_pool(name="w", bufs=DC))
        p_wo = ctx.enter_context(tc.tile_pool(name="wo", bufs=2))
        p_qk = ctx.enter_context(tc.tile_pool(name="qk", bufs=2))
        p_v = ctx.enter_context(tc.tile_pool(name="v", bufs=S // JC))
        p_mg = ctx.enter_context(tc.tile_pool(name="mg", bufs=2))
        p_probs = ctx.enter_context(tc.tile_pool(name="probs", bufs=8))
        p_small = ctx.enter_context(tc.tile_pool(name="small", bufs=2))
        p_mask = ctx.enter_context(tc.tile_pool(name="mask", bufs=1))
        p_ostg = ctx.enter_context(tc.tile_pool(name="ostg", bufs=2))
        p_ones = ctx.enter_context(tc.tile_pool(name="ones", bufs=1))

        ps_big = ctx.enter_context(tc.tile_pool(name="psb", bufs=2, space="PSUM"))
        ps_sc = ctx.enter_context(tc.tile_pool(name="pss", bufs=2, space="PSUM"))
        ps_at = ctx.enter_context(tc.tile_pool(name="psa", bufs=2, space="PSUM"))

        # ---- HAM pre-warm: the PE idles waiting for the first x/w tiles
        # anyway; a burst of discarded fp32 matmuls keeps the activity
        # monitor busy so the clock gate is at full rate when the real
        # projections start.
        wrm = p_ones.tile([128, 512], F32, tag="warm")
        nc.vector.memset(wrm[:], 1.0)
        wrmb = p_ones.tile([128, 512], BF16, tag="warmb")
        nc.vector.memset(wrmb[:], 1.0)
        wrm_ps = ps_at.tile([128, 512], F32, tag="attn", name="warmps")
        for r in range(9):
            nc.tensor.matmul(wrm_ps[:], wrm[:, 0:128], wrm[:],
                             start=(r == 0), stop=(r == 8))
        nc.scalar.copy(wrm[:, 0:1], wrm_ps[:, 0:1])  # keep alive vs DCE

        # ---- input loads, in consumption order
        w_t, x_t = [], []
        for dc in range(DC):
            eng = nc.gpsimd if dc < 2 else nc.sync
            wt = p_w.tile([128, 3 * M], BF16, tag="w")
            eng.dma_start(wt[:], wqkv_d[dc * 128:(dc + 1) * 128, :])
            w_t.append(wt)
            xt = p_x.tile([128, S], BF16, tag="x")
            eng.dma_start(xt[:], xT_d[dc * 128:(dc + 1) * 128, :])
            x_t.append(xt)
        wo_t = []
        for kc in range(2):
            t = p_wo.tile([128, D], BF16, tag="wo")
            nc.sync.dma_start(t[:], woT_d[kc * 128:(kc + 1) * 128, :])
            wo_t.append(t)
        mask_t = p_mask.tile([JC, 768], BF16, tag="mask")
        nc.sync.dma_start(mask_t[:], mask_d[:])
        onesb_t = p_ones.tile([JC, HPG], BF16, tag="onesb")
        nc.sync.dma_start(onesb_t[:], onesb_d[:])

        # ---- projections ----
        q_t, k_t = {}, {}

        def qk_proj0():
            # m-chunk 0 runs while the input DMA is still streaming: the
            # d-contraction is split in two half-groups (dc 0-3, dc 4-7)
            # merged at eviction, so the in-order PE never stalls waiting
            # for the last x tiles.
            for woff, store, tg in ((0, q_t, "qT"), (M, k_t, "kT")):
                dst = p_qk.tile([128, S], BF16, tag=tg, name=f"{tg}0")
                for s4 in range(S // 512):
                    sl = slice(s4 * 512, (s4 + 1) * 512)
                    psa = ps_big.tile([128, 512], F32, tag="proj")
                    for dc in range(DC // 2):
                        nc.tensor.matmul(
                            psa[:], w_t[dc][:, woff:woff + 128], x_t[dc][:, sl],
                            start=(dc == 0), stop=(dc == DC // 2 - 1))
                    psb = ps_sc.tile([128, 2 * IC], F32, tag="scores")
                    for dc in range(DC // 2, DC):
                        nc.tensor.matmul(
                            psb[:, 0:512], w_t[dc][:, woff:woff + 128],
                            x_t[dc][:, sl],
                            start=(dc == DC // 2), stop=(dc == DC - 1))
                    nc.scalar.copy(dst[:, sl], psa[:])
                    nc.vector.tensor_add(dst[:, sl], dst[:, sl], psb[:, 0:512])
                store[0] = dst

        def qk1_unit(which, s4):
            # m-chunk 1, one (q|k, s4) block: inputs have long arrived, so a
            # single 8-deep accumulation chain + one ACT eviction suffices.
            woff, store, tg = ((0, q_t, "qT") if which == "q" else (M, k_t, "kT"))
            if 1 not in store:
                store[1] = p_qk.tile([128, S], BF16, tag=tg, name=f"{tg}1")
            dst = store[1]
            sl = slice(s4 * 512, (s4 + 1) * 512)
            psa = ps_big.tile([128, 512], F32, tag="proj")
            for dc in range(DC):
                nc.tensor.matmul(
                    psa[:], w_t[dc][:, woff + 128:woff + 256], x_t[dc][:, sl],
                    start=(dc == 0), stop=(dc == DC - 1))
            nc.scalar.copy(dst[:, sl], psa[:])

        v_t = {}

        def v_proj(sc, split):
            # v[s, m] tile for j-chunk sc: per head h cols h*65..h*65+63 = v,
            # col h*65+64 = 1.0 (softmax denominator column).
            vt = p_v.tile([JC, HPG * (DH + 1)], BF16, tag="v", name=f"v{sc}")
            nc.vector.tensor_copy(
                vt[:].rearrange("p (h e) -> p h e", h=HPG)[:, :, DH:DH + 1].squeeze(2),
                onesb_t[:])
            dstv = vt[:].rearrange("p (h e) -> p h e", h=HPG)[:, :, 0:DH]
            if split:
                psa = ps_big.tile([128, 512], F32, tag="proj")
                for dc in range(DC // 2):
                    nc.tensor.matmul(
                        psa[:, 0:M], x_t[dc][:, sc * 128:(sc + 1) * 128],
                        w_t[dc][:, 2 * M:3 * M],
                        start=(dc == 0), stop=(dc == DC // 2 - 1))
                psb = ps_sc.tile([128, 2 * IC], F32, tag="scores")
                for dc in range(DC // 2, DC):
                    nc.tensor.matmul(
                        psb[:, 0:M], x_t[dc][:, sc * 128:(sc + 1) * 128],
                        w_t[dc][:, 2 * M:3 * M],
                        start=(dc == DC // 2), stop=(dc == DC - 1))
                nc.scalar.activation(
                    dstv, psa[:, 0:M].rearrange("p (h d) -> p h d", h=HPG),
                    mybir.ActivationFunctionType.Copy)
                nc.vector.tensor_add(
                    dstv, dstv, psb[:, 0:M].rearrange("p (h d) -> p h d", h=HPG))
            else:
                psa = ps_big.tile([128, 512], F32, tag="proj")
                for dc in range(DC):
                    nc.tensor.matmul(
                        psa[:, 0:M], x_t[dc][:, sc * 128:(sc + 1) * 128],
                        w_t[dc][:, 2 * M:3 * M],
                        start=(dc == 0), stop=(dc == DC - 1))
                nc.vector.tensor_copy(
                    dstv, psa[:, 0:M].rearrange("p (h d) -> p h d", h=HPG))
            v_t[sc] = vt

        # ---- attention ----
        mg_t = [p_mg.tile([128, S], BF16, tag="mgT", name=f"mg{i}")
                for i in range(M // 128)]
        gstate = {}  # (h, ic) -> {"at": psum tile, "prs": [(pr, ja, jb)]}

        def gs(h, ic):
            return gstate.setdefault((h, ic), {"at": None, "prs": []})

        def at_tile(g):
            if g["at"] is None:
                g["at"] = ps_at.tile([DH + 1, IC], F32, tag="attn", name="atps")
            return g["at"]

        def sc_block(h, ic, pa, pb):
            # scores+exp for non-diagonal pairs pa..pb-1 of group (h, ic)
            g = gs(h, ic)
            qk_tile, prow = h // 2, 64 * (h % 2)
            kk, qq = k_t[qk_tile], q_t[qk_tile]
            i0 = ic * IC
            for p in range(pa, pb):
                ja, jb = 2 * p, 2 * p + 2
                sc_ps = ps_sc.tile([128, 2 * IC], F32, tag="scores")
                pr = p_probs.tile([JC, 2 * IC], BF16, tag="probs")
                for u, jc in enumerate(range(ja, jb)):
                    nc.tensor.matmul(
                        sc_ps[:, u * IC:(u + 1) * IC],
                        kk[prow:prow + DH, jc * JC:(jc + 1) * JC],
                        qq[prow:prow + DH, i0:i0 + IC],
                        start=True, stop=True)
                nc.scalar.activation(pr[:], sc_ps[:], EXP, scale=SCALE)
                g["prs"].append((pr, ja, jb))

        def av_block(h, ic):
            # consume all pending probs of group (h, ic)
            g = gs(h, ic)
            at_ps = at_tile(g)
            vsl = slice(h * (DH + 1), (h + 1) * (DH + 1))
            for pr, ja, jb in g["prs"]:
                for u, jc in enumerate(range(ja, jb)):
                    nc.tensor.matmul(
                        at_ps[:], v_t[jc][:, vsl], pr[:, u * IC:(u + 1) * IC],
                        start=(jc == 0), stop=False)
            g["prs"] = []

        def diag_scores(h, ic):
            # diagonal chunks nd..nd+3: d0 full 512 stream, d128 384 (cols
            # 512:896 of pair A), d256/d384 256 each (pair B cols 0:512).
            g = gs(h, ic)
            qk_tile, prow = h // 2, 64 * (h % 2)
            kk, qq = k_t[qk_tile], q_t[qk_tile]
            i0 = ic * IC
            nd = 4 * ic
            psA = ps_sc.tile([128, 2 * IC], F32, tag="scores")
            prA = p_probs.tile([JC, 2 * IC], BF16, tag="probs")
            nc.tensor.matmul(
                psA[:, 0:512], kk[prow:prow + DH, nd * JC:(nd + 1) * JC],
                qq[prow:prow + DH, i0:i0 + IC], start=True, stop=True)
            nc.tensor.matmul(
                psA[:, 512:896], kk[prow:prow + DH, (nd + 1) * JC:(nd + 2) * JC],
                qq[prow:prow + DH, i0 + 128:i0 + IC], start=True, stop=True)
            psB = ps_sc.tile([128, 2 * IC], F32, tag="scores")
            prB = p_probs.tile([JC, 2 * IC], BF16, tag="probs")
            nc.tensor.matmul(
                psB[:, 0:256], kk[prow:prow + DH, (nd + 2) * JC:(nd + 3) * JC],
                qq[prow:prow + DH, i0 + 256:i0 + IC], start=True, stop=True)
            nc.tensor.matmul(
                psB[:, 256:512], kk[prow:prow + DH, (nd + 3) * JC:(nd + 4) * JC],
                qq[prow:prow + DH, i0 + 256:i0 + IC], start=True, stop=True)
            nc.scalar.activation(prA[:, 0:896], psA[:, 0:896], EXP, scale=SCALE)
            nc.scalar.activation(prB[:, 0:512], psB[:, 0:512], EXP, scale=SCALE)
            pvA = prA[:].rearrange("p (a b) -> p a b", a=2)[:, :, 0:128]
            nc.vector.tensor_mul(
                pvA, pvA, mask_t[:, 0:256].rearrange("p (a b) -> p a b", a=2))
            nc.vector.tensor_mul(prB[:, 0:512], prB[:, 0:512], mask_t[:, 256:768])
            g["diag"] = (prA, prB)

        def diag_avs(h, ic):
            g = gs(h, ic)
            at_ps = at_tile(g)
            prA, prB = g.pop("diag")
            vsl = slice(h * (DH + 1), (h + 1) * (DH + 1))
            nd = 4 * ic
            nc.tensor.matmul(at_ps[:], v_t[nd][:, vsl], prA[:, 0:512],
                             start=(nd == 0), stop=False)
            nc.tensor.matmul(at_ps[:, 128:512], v_t[nd + 1][:, vsl],
                             prA[:, 512:896], start=False, stop=False)
            nc.tensor.matmul(at_ps[:, 256:512], v_t[nd + 2][:, vsl],
                             prB[:, 0:256], start=False, stop=False)
            nc.tensor.matmul(at_ps[:, 256:512], v_t[nd + 3][:, vsl],
                             prB[:, 256:512], start=False, stop=True)

        def normalize(h, ic):
            # DVE copies the denominator row to SBUF, reciprocates it,
            # GPSIMD broadcasts the reciprocal across 64 partitions, DVE
            # multiplies during the attnT eviction into mergedT.
            g = gstate.pop((h, ic))
            at_ps = g["at"]
            qk_tile, prow = h // 2, 64 * (h % 2)
            den = p_small.tile([1, IC], F32, tag="den")
            nc.vector.tensor_copy(den[:], at_ps[DH:DH + 1, :])
            rc32 = p_small.tile([1, IC], F32, tag="recip32")
            nc.vector.reciprocal_approx_fast(rc32[:], den[:])
            bc_sb = p_small.tile([DH, IC], F32, tag="bcast")
            nc.gpsimd.partition_broadcast(bc_sb[:], rc32[:])
            nc.vector.tensor_mul(
                mg_t[qk_tile][prow:prow + DH, ic * IC:(ic + 1) * IC],
                at_ps[0:DH, :], bc_sb[:])

        def oproj(sc, act_half=False):
            # out[s, o] = sum_k mergedT[k, s] woT[k, o]; both half-evictions
            # on DVE while ACT is saturated with exps; the closing blocks
            # (no exps left) put one half on the otherwise-idle ACT instead.
            stg = p_ostg.tile([128, D], F16, tag="ostg")
            for nn in range(2):
                ps = ps_big.tile([128, 512], F32, tag="proj")
                for kc in range(2):
                    nc.tensor.matmul(
                        ps[:], mg_t[kc][:, sc * 128:(sc + 1) * 128],
                        wo_t[kc][:, nn * 512:(nn + 1) * 512],
                        start=(kc == 0), stop=(kc == 1))
                if act_half and nn == 1:
                    nc.scalar.copy(stg[:, 512:1024], ps[:])
                else:
                    nc.vector.tensor_copy(stg[:, nn * 512:(nn + 1) * 512], ps[:])
            nc.sync.dma_start(out_d[sc * 128:(sc + 1) * 128, :], stg[:])

        # ---- emission schedule ----
        # ic=0 groups are diagonal-only; qk m-chunk 1 units fill the PE
        # stream between them. v chunks and o-projection blocks are woven
        # into later attention groups so the PE always has independent work
        # queued while ACT drains the exp backlog of the previous block.
        qk_proj0()
        for sc in range(4):
            v_proj(sc, split=True)

        OPS = [
            ("dgs", 0, 0), ("dga", 0, 0),
            ("q1", "q", 0),
            ("dgs", 1, 0), ("dga", 1, 0),
            ("nrm", 0, 0),
            ("q1", "q", 1), ("q1", "q", 2), ("q1", "q", 3),
            ("nrm", 1, 0),
            ("q1", "k", 0), ("q1", "k", 1), ("q1", "k", 2), ("q1", "k", 3),
            ("dgs", 2, 0), ("dga", 2, 0),
            ("v", 4),
            ("dgs", 3, 0), ("dga", 3, 0),
            ("nrm", 2, 0),
            ("v", 5),
            ("nrm", 3, 0),
            ("op", 0), ("v", 6), ("op", 1), ("v", 7), ("op", 2), ("op", 3),
            # ---- ic = 1: one 2-pair block + diagonal per group ----
            ("scb", 0, 1, 0, 2), ("v", 8), ("avb", 0, 1), ("dgs", 0, 1), ("dga", 0, 1),
            ("scb", 1, 1, 0, 2), ("v", 9), ("avb", 1, 1), ("dgs", 1, 1), ("dga", 1, 1),
            ("nrm", 0, 1),
            ("scb", 2, 1, 0, 2), ("v", 10), ("avb", 2, 1), ("dgs", 2, 1), ("dga", 2, 1),
            ("nrm", 1, 1),
            ("scb", 3, 1, 0, 2), ("v", 11), ("avb", 3, 1), ("dgs", 3, 1), ("dga", 3, 1),
            ("nrm", 2, 1),
            # ---- ic = 2: two 2-pair blocks + diagonal per group ----
            ("scb", 0, 2, 0, 2), ("nrm", 3, 1), ("avb", 0, 2),
            ("scb", 0, 2, 2, 4), ("v", 12), ("avb", 0, 2), ("dgs", 0, 2), ("dga", 0, 2),
            ("scb", 1, 2, 0, 2), ("v", 13), ("avb", 1, 2),
            ("scb", 1, 2, 2, 4), ("v", 14), ("avb", 1, 2), ("dgs", 1, 2), ("dga", 1, 2),
            ("nrm", 0, 2),
            ("scb", 2, 2, 0, 2), ("v", 15), ("avb", 2, 2),
            ("scb", 2, 2, 2, 4), ("op", 4), ("avb", 2, 2), ("dgs", 2, 2), ("dga", 2, 2),
            ("nrm", 1, 2),
            ("scb", 3, 2, 0, 2), ("op", 5), ("avb", 3, 2),
            ("scb", 3, 2, 2, 4), ("op", 6), ("avb", 3, 2), ("dgs", 3, 2), ("dga", 3, 2),
            ("nrm", 2, 2),
            # ---- ic = 3: three 2-pair blocks + diagonal per group ----
            ("scb", 0, 3, 0, 2), ("op", 7), ("avb", 0, 3),
            ("scb", 0, 3, 2, 4), ("nrm", 3, 2), ("avb", 0, 3),
            ("scb", 0, 3, 4, 6), ("avb", 0, 3), ("dgs", 0, 3), ("dga", 0, 3),
            ("scb", 1, 3, 0, 2), ("avb", 1, 3),
            ("scb", 1, 3, 2, 4), ("avb", 1, 3),
            ("scb", 1, 3, 4, 6), ("avb", 1, 3), ("dgs", 1, 3), ("dga", 1, 3),
            ("nrm", 0, 3),
            ("scb", 2, 3, 0, 2), ("op", 8), ("avb", 2, 3),
            ("scb", 2, 3, 2, 4), ("op", 9), ("avb", 2, 3),
            ("scb", 2, 3, 4, 6), ("op", 10), ("avb", 2, 3), ("dgs", 2, 3), ("dga", 2, 3),
            ("nrm", 1, 3),
            ("scb", 3, 3, 0, 2), ("op", 11), ("avb", 3, 3),
            ("scb", 3, 3, 2, 4), ("nrm", 2, 3), ("avb", 3, 3),
            ("scb", 3, 3, 4, 6), ("avb", 3, 3), ("dgs", 3, 3), ("dga", 3, 3),
            ("twarm",), ("nrm", 3, 3),
            ("op", 12), ("op", 13), ("op", 14), ("op", 15),
        ]
        for op in OPS:
            kind = op[0]
            if kind == "q1":
                qk1_unit(op[1], op[2])
            elif kind == "v":
                v_proj(op[1], split=False)
            elif kind == "scb":
                sc_block(op[1], op[2], op[3], op[4])
            elif kind == "avb":
                av_block(op[1], op[2])
            elif kind == "dgs":
                diag_scores(op[1], op[2])
            elif kind == "dga":
                diag_avs(op[1], op[2])
            elif kind == "nrm":
                normalize(op[1], op[2])
            elif kind == "op":
                oproj(op[1], act_half=(op[1] >= 12))
            elif kind == "twarm":
                # dummy burst bridging the final normalize's latency window:
                # keeps the HAM activity monitor fed so the closing
                # o-projections run at full clock instead of k=4/8. Allocated
                # from the ps_big ring (ps_at's buffers are still being read
                # by the pending normalizes at this point).
                twps = ps_big.tile([128, 512], F32, tag="proj", name="twps")
                for r in range(14):
                    nc.tensor.matmul(twps[:], wrmb[:, 0:128], wrmb[:],
                                     start=(r == 0), stop=(r == 13))
                nc.scalar.copy(wrm[:, 1:2], twps[:, 0:1])


_NC_CACHE = None


def _get_nc():
    global _NC_CACHE
    if _NC_CACHE is None:
        _NC_CACHE = _build_nc()
    return _NC_CACHE


def _causal_mask_tile():
    # [128, 768]: cols 0:128 and 128:256 = tri(j <= u) (masks the first 128
    # trimmed columns of the d0 and d128 diagonal chunks via one strided
    # multiply); cols 256:512 = tri(j <= u) over u in [0,256) (d256); cols
    # 512:768 = tri(j <= u - 128) (d384).
    j = np.arange(JC)[:, None]
    u128 = np.arange(128)[None, :]
    u256 = np.arange(256)[None, :]
    tri128 = (j <= u128).astype(np.float32)
    triB1 = (j <= u256).astype(np.float32)
    triB2 = (j <= u256 - 128).astype(np.float32)
    return np.concatenate([tri128, tri128, triB1, triB2], axis=1)


def _prepare_in_maps(inputs):
    bf = ml_dtypes.bfloat16
    x = np.asarray(inputs["in_features"], dtype=np.float32)
    wqT = np.ascontiguousarray(np.asarray(inputs["q_proj_weight"], np.float32).T)
    wkT = np.ascontiguousarray(np.asarray(inputs["k_proj_weight"], np.float32).T)
    wvT = np.ascontiguousarray(np.asarray(inputs["v_proj_weight"], np.float32).T)
    woT = np.ascontiguousarray(np.asarray(inputs["o_proj_weight"], np.float32).T)
    xT = [np.ascontiguousarray(x[b].T) for b in range(B)]
    mask = _causal_mask_tile()

    in_maps = []
    for c in range(NCORES):
        b, g = divmod(c, HPG)
        ms = slice(g * M, (g + 1) * M)
        in_maps.append({
            "xT": xT[b].astype(bf),
            "wqkvT": np.ascontiguousarray(
                np.concatenate([wqT[:, ms], wkT[:, ms], wvT[:, ms]], axis=1)).astype(bf),
            "woT": np.ascontiguousarray(woT[ms, :]).astype(bf),
            "mask": mask.astype(bf),
            "ones_b": np.ones((JC, HPG), bf),
        })
    return in_maps


def kernel(q_proj_weight, k_proj_weight, v_proj_weight, o_proj_weight, in_features):
    in_dtype = np.asarray(in_features).dtype
    in_maps = _prepare_in_maps({
        "q_proj_weight": q_proj_weight,
        "k_proj_weight": k_proj_weight,
        "v_proj_weight": v_proj_weight,
        "o_proj_weight": o_proj_weight,
        "in_features": in_features,
    })
    nc = _get_nc()
    res = bass_utils.run_bass_kernel_spmd(nc, in_maps, core_ids=list(range(NCORES)))
    out = np.zeros((B, S, D), dtype=np.float32)
    for c in range(NCORES):
        out[c // HPG] += res.results[c]["out"]
    return out.astype(in_dtype)
